# revision 14
# baseline (speedup 1.0000x reference)
"""Bilateral filter (nn_BilateralFilter) Trainium2 Bass kernel, v2.

Reference semantics (KERNEL_SIZE=5, THETA_ALPHA=2.0, THETA_BETA=0.1):
    w_k   = exp(-(dx^2+dy^2)/8)                      (24 offsets, center dropped)
    Ki    = exp(-50*(I(p+k) - I(p))^2)               per image channel c
    out[c,n,p] = sum_k w_k*Ki[c,k,p]*Q(n,p+k) / sum_k w_k*Ki[c,k,p]

v2 strategy (vs v1's all-DVE fold at 199us):
  The range kernel is symmetric in its two endpoints, so
  Ki_k(p) = Ki_{-k}(p+k) and therefore
      Qtilde(p) = sum_j U_j(p - j),   U_j(q) = w_j*Ki_j(q)*Q(q)
  i.e. every product is SAME-SITE (no shifted Q reads), and the 24-tap
  shifted fold becomes 24 one-hot-shift matmuls accumulated in fp32 PSUM
  on the otherwise-idle TensorEngine.  Per core: partitions = (c=3, y=42)
  over two U-row groups [-2,40) and [40,82); row shifts live in the
  host-precomputed 0/1 stationaries (with 2-row cross-group halo
  matmuls), column shifts are free-dim offsets of the moving operand.
  DVE keeps only subs, the 24 U-products (fp16, 2x mode), reciprocal and
  the final PSUM*1/norm; ACT does Square and per-slot Exp with the full
  spatial weight folded into the exp bias (+SHIFT trick cancels in the
  division).  norm(p) = sum_k w_k Ki_k(p) rides the same PSUM path with
  unshifted stationaries.
Sharding: 8 cores = 2 batches x 4 row-slabs of 80 output rows.
"""

import numpy as np
import ml_dtypes

B, C, NCL = 2, 3, 6
H = W = 320
KS, PAD = 5, 2
NSLAB = 4
R = H // NSLAB            # 80 output rows per shard
COEF = 50.0               # 1/(2*theta_beta^2)
SHIFT = 8.0               # exponent shift, cancels in the division
YG = 42                   # U-rows per group ([-2,40) and [40,82))
NP_ = 3 * YG              # 126 partitions for U/kw tiles
MP = 3 * 40               # 120 output partitions (c, y40)
XI = W + 8                # 328: I cols incl +-4 pad
XU = W + 4                # 324: U/kw cols incl +-2 pad
IR = R + 8                # 88 I rows per core
QR = R + 4                # 84 Q rows per core
NST = 14                  # stationaries

_CACHE: dict = {}


def _stationaries():
    """14 one-hot stationaries [126, 14*120] fp16, built on host.

    Slot map (dr = raw 0..4, dr' = dr-2):
      0..4  : S_A_dr   main grp A: p=(c, py=y+2-dr'), m=(c,y), py<42
      5..6  : S_Ah_dr  halo  grp A (dr=0,1): p=(c, pyB), U^B rows 40,41
      7..11 : S_B_dr   main grp B: p=(c, py=my-dr'),  m=(c,my), py>=0
      12..13: S_Bh_dr  halo  grp B (dr=3,4): p=(c, pyA), U^A rows 38,39
    """
    s = np.zeros((NP_, NST * MP), np.float32)

    def S(idx):
        return s[:, idx * MP:(idx + 1) * MP].reshape(NP_, 3, 40)

    for dr in range(5):
        drp = dr - 2
        sa = S(dr)
        sb = S(7 + dr)
        for c in range(3):
            for y in range(40):
                py = y + 2 - drp
                if 0 <= py < YG:
                    sa[c * YG + py, c, y] = 1.0
                py = y - drp
                if 0 <= py < YG:
                    sb[c * YG + py, c, y] = 1.0
    # grp A halo: U rows q=40,41 live in tile B partitions 0,1
    for i, dr in enumerate((0, 1)):
        sh = S(5 + i)
        for c in range(3):
            for pyb in range(2):
                y = 40 + pyb + dr - 2
                if 0 <= y < 40:
                    sh[c * YG + pyb, c, y] = 1.0
    # grp B halo: U rows q=38,39 live in tile A partitions 40,41
    for i, dr in enumerate((3, 4)):
        sh = S(12 + i)
        for c in range(3):
            for pya in (40, 41):
                q = pya - 2
                my = q + (dr - 2) - 40
                if 0 <= my < 40:
                    sh[c * YG + pya, c, my] = 1.0
    return s.astype(ml_dtypes.bfloat16)


def _emit(tc, i_ap, q_ap, s_ap, out_ap):
    """i_ap: (264, 328) fp16   rows = (c, 88)
    q_ap: (84, 1944) fp16     rows = U-rows [-2,82), cols (n,324)
    s_ap: (126, 1680) fp16    stationaries
    out_ap: (80, 5760) fp16   cols (c, n, x)
    """
    import concourse.bass as bass
    import concourse.mybir as mybir

    f16 = mybir.dt.float16
    bf16 = mybir.dt.bfloat16
    f32 = mybir.dt.float32
    AF = mybir.ActivationFunctionType
    nc = tc.nc
    SLOTS = [(dr, dc) for dr in (2, 0, 1, 3, 4) for dc in range(5)
             if not (dr == 2 and dc == 2)]
    BIAS = {
        (dr, dc): SHIFT - ((dr - 2) ** 2 + (dc - 2) ** 2) / 8.0
        for dr, dc in SLOTS
    }

    with tc.tile_pool(name="p", bufs=1) as pool, \
            tc.psum_pool(name="pp", bufs=1) as ppool:
        stat = pool.tile([NP_, NST * MP], bf16, tag="stat")

        distinct = sorted(set(BIAS.values()))
        bcol = {v: j for j, v in enumerate(distinct)}
        bias_t = pool.tile([NP_, len(distinct)], mybir.dt.float32, tag="bias")
        for v, j in bcol.items():
            nc.vector.memset(bias_t[:, j:j + 1], v)

        def st(idx):
            return stat[:, idx * MP:(idx + 1) * MP]

        # per-(grp, dr) shifted I copies [(c,42), 328], center row-shift
        # first so the first subs unblock early; g1 loads ride the GpSimd
        # DGE so the two groups' input streams issue in parallel
        Idr = {}
        for g in range(2):
            for dr in (2, 0, 1, 3, 4):
                t = pool.tile([NP_, XI], f16, tag=f"I{g}{dr}")
                eng = nc.sync if g == 0 else nc.gpsimd
                for c in range(3):
                    eng.dma_start(
                        t[c * YG:(c + 1) * YG, :],
                        i_ap[c * IR + g * YG + dr: c * IR + g * YG + dr + YG, :],
                    )
                Idr[(g, dr)] = t
        # Q replicated over c: [(c,42), (n,324)] per grp
        Qrep = []
        for g in range(2):
            t = pool.tile([NP_, NCL * XU], f16, tag=f"Q{g}")
            for c in range(3):
                nc.scalar.dma_start(
                    t[c * YG:(c + 1) * YG, :],
                    q_ap[g * YG:(g + 1) * YG, :],
                )
            Qrep.append(t)
        nc.scalar.dma_start(stat[:, :], s_ap[:, :])

        # kw[(c,42), (slot25, 324)]: d -> square -> exp(in place);
        # dr=2 first (its I tiles DMA first) and exps interleaved right
        # after each square so the first norm matmuls unblock early
        DRS = (2, 0, 1, 3, 4)
        kw = []
        for g in range(2):
            kwg = pool.tile([NP_, 25 * XU], bf16, tag=f"kw{g}")
            d = pool.tile([NP_, 5 * XU], f16, tag=f"d{g}", bufs=2)
            for dr in DRS:
                # d[dc, x] = I(q + (dr', dc')) - I(q); all 5 dc in one op
                in0 = bass.AP(
                    tensor=Idr[(g, dr)].tensor, offset=Idr[(g, dr)].offset,
                    ap=[[XI, NP_], [1, 5], [1, XU]],
                )
                in1 = bass.AP(
                    tensor=Idr[(g, 2)].tensor, offset=Idr[(g, 2)].offset + 2,
                    ap=[[XI, NP_], [0, 5], [1, XU]],
                )
                dst = d[:, :].rearrange("p (dc x) -> p dc x", dc=5)
                nc.vector.tensor_sub(dst, in0, in1)
                nc.scalar.activation(
                    kwg[:, dr * 5 * XU:(dr + 1) * 5 * XU], d[:, :], AF.Square
                )
                for dc in range(5):
                    if (dr, dc) == (2, 2):
                        continue
                    sl = dr * 5 + dc
                    nc.scalar.activation(
                        kwg[:, sl * XU:(sl + 1) * XU],
                        kwg[:, sl * XU:(sl + 1) * XU],
                        AF.Exp, bias=bias_t[:, bcol[BIAS[(dr, dc)]]:
                                            bcol[BIAS[(dr, dc)]] + 1],
                        scale=-COEF,
                    )
            kw.append(kwg)

        # PSUM: per grp 4 fbanks of (n6, x80) + 1 norm bank
        NFB = 4
        XB = W // NFB  # 80
        qt = [ppool.tile([MP, NCL * XB], f32, tag=f"qt{b}", name=f"qt{b}")
              for b in range(NFB)]
        nrm = ppool.tile([MP, W], f32, tag="nrm", name="nrm")

        def u_prod(g, dr, dc, tag, bufs=1):
            sl = dr * 5 + dc
            u = pool.tile([NP_, NCL * XU], bf16, tag=tag, bufs=bufs)
            kws = bass.AP(
                tensor=kw[g].tensor, offset=kw[g].offset + sl * XU,
                ap=[[25 * XU, NP_], [0, NCL], [1, XU]],
            )
            qs = bass.AP(
                tensor=Qrep[g].tensor, offset=Qrep[g].offset,
                ap=[[NCL * XU, NP_], [XU, NCL], [1, XU]],
            )
            nc.vector.tensor_mul(
                u[:, :].rearrange("p (n x) -> p n x", n=NCL), kws, qs
            )
            return u

        def mm_u(g, u, dr, dc, sidx, start, stop):
            for b in range(NFB):
                mv = bass.AP(
                    tensor=u.tensor, offset=u.offset + (4 - dc) + b * XB,
                    ap=[[NCL * XU, NP_], [XU, NCL], [1, XB]],
                )
                nc.tensor.matmul(
                    qt[b][:, :], st(sidx), mv,
                    start=start, stop=stop,
                )

        def mm_norm(g, dr, dc, start, stop):
            sl = dr * 5 + dc
            mv = bass.AP(
                tensor=kw[g].tensor, offset=kw[g].offset + sl * XU + 2,
                ap=[[25 * XU, NP_], [1, W]],
            )
            nc.tensor.matmul(
                nrm[:, :], st(2 if g == 0 else 9), mv,
                start=start, stop=stop,
            )

        # ---- grp A: norm MMs first (only need kw), then main ----
        for i, (dr, dc) in enumerate(SLOTS):
            mm_norm(0, dr, dc, start=i == 0, stop=i == len(SLOTS) - 1)
        upinA = {}
        first = True
        for dr, dc in SLOTS:
            if dr >= 3:
                u = u_prod(0, dr, dc, tag=f"up{dr}{dc}")
                upinA[(dr, dc)] = u
            else:
                u = u_prod(0, dr, dc, tag="urot", bufs=8)
            mm_u(0, u, dr, dc, dr, start=first, stop=False)
            first = False
        # ---- grp B products for dr 0,1 feed grp A halo ----
        uB01 = {}
        for dr in range(2):
            for dc in range(5):
                u = u_prod(1, dr, dc, tag=f"uB{dr}{dc}")
                uB01[(dr, dc)] = u
                last = (dr, dc) == (1, 4)
                mm_u(0, u, dr, dc, 5 + dr, start=False, stop=last)

        # ---- grp A finals: rnorm + PSUM*rnorm -> out ----
        def finals(g):
            rn = pool.tile([MP, W], f32, tag=f"rn{g}")
            nc.vector.reciprocal_approx_fast(rn[:, :], nrm[:, :])
            ot = pool.tile([MP, NCL * W], f16, tag=f"ot{g}")
            for b in range(NFB):
                dst = bass.AP(
                    tensor=ot.tensor, offset=ot.offset + b * XB,
                    ap=[[NCL * W, MP], [W, NCL], [1, XB]],
                )
                rnb = bass.AP(
                    tensor=rn.tensor, offset=rn.offset + b * XB,
                    ap=[[W, MP], [0, NCL], [1, XB]],
                )
                src = qt[b][:, :].rearrange("p (n x) -> p n x", n=NCL)
                nc.vector.tensor_mul(dst, src, rnb)
            for c in range(3):
                nc.sync.dma_start(
                    out_ap[g * 40:(g + 1) * 40,
                           c * NCL * W:(c + 1) * NCL * W],
                    ot[c * 40:(c + 1) * 40, :],
                )

        # grp B's remaining U products issue before finals(0) on the DVE
        # queue; their MMs still wait on finals(0) freeing the PSUM banks.
        finals(0)

        uB = dict(uB01)
        for dr in (2, 3, 4):
            for dc in range(5):
                if (dr, dc) == (2, 2):
                    continue
                uB[(dr, dc)] = u_prod(1, dr, dc, tag="urot", bufs=8)

        # ---- grp B: norm MMs, then main; halo from pinned U^A ----
        for i, (dr, dc) in enumerate(SLOTS):
            mm_norm(1, dr, dc, start=i == 0, stop=i == len(SLOTS) - 1)
        first = True
        for dr, dc in SLOTS:
            mm_u(1, uB[(dr, dc)], dr, dc, 7 + dr, start=first, stop=False)
            first = False
        for i, dr in enumerate((3, 4)):
            for dc in range(5):
                last = (dr, dc) == (4, 4)
                mm_u(1, upinA[(dr, dc)], dr, dc, 12 + i,
                     start=False, stop=last)

        finals(1)


def _build_program():
    import concourse.bacc as bacc
    import concourse.mybir as mybir
    from concourse import tile

    f16 = mybir.dt.float16

    nc = bacc.Bacc("TRN2", num_devices=8, debug=False)
    I_in = nc.dram_tensor("i_in", [3 * IR, XI], f16, kind="ExternalInput")
    Q_in = nc.dram_tensor("q_in", [QR, NCL * XU], f16, kind="ExternalInput")
    S_in = nc.dram_tensor("s_in", [NP_, NST * MP], mybir.dt.bfloat16,
                          kind="ExternalInput")
    OUT = nc.dram_tensor("out", [R, C * NCL * W], f16, kind="ExternalOutput")

    with tile.TileContext(nc) as tc:
        _emit(tc, I_in.ap(), Q_in.ap(), S_in.ap(), OUT.ap())

    nc.compile()
    return nc


def _get_program():
    if "nc" not in _CACHE:
        _CACHE["nc"] = _build_program()
    return _CACHE["nc"]


def _shard_inputs(Q, I):
    """Host prep: pad, cast fp16, per-shard layouts (see _emit)."""
    Qp = np.pad(
        np.asarray(Q, np.float32), ((0, 0), (0, 0), (PAD, PAD), (PAD, PAD))
    ).astype(np.float16)
    Ip = np.pad(
        np.asarray(I, np.float32), ((0, 0), (0, 0), (4, 4), (4, 4))
    ).astype(np.float16)
    s_np = _stationaries()
    in_maps = []
    for b in range(B):
        for s in range(NSLAB):
            r0 = s * R
            i_sh = Ip[b, :, r0:r0 + IR, :]            # (C, 88, 328)
            q_sh = Qp[b, :, r0:r0 + QR, :]            # (NCL, 84, 324)
            in_maps.append(
                {
                    "i_in": np.ascontiguousarray(i_sh.reshape(3 * IR, XI)),
                    "q_in": np.ascontiguousarray(
                        q_sh.transpose(1, 0, 2).reshape(QR, NCL * XU)
                    ),
                    "s_in": s_np,
                }
            )
    return in_maps


def _assemble(outs):
    # outs: list of 8 arrays (R, C*NCL*W), core order = (b, slab)
    o = np.stack([np.asarray(x) for x in outs]).astype(np.float32)
    o = o.reshape(B, NSLAB, R, C, NCL, W)
    o = o.transpose(0, 3, 4, 1, 2, 5).reshape(B, C, NCL, H, W)
    return o


def run(Q, I, trace=False):
    from concourse.bass_utils import run_bass_kernel_spmd

    nc = _get_program()
    in_maps = _shard_inputs(Q, I)
    res = run_bass_kernel_spmd(nc, in_maps, list(range(8)), trace=trace)
    out = _assemble([res.results[i]["out"] for i in range(8)])
    return out, res


def kernel(Q, I):
    out, _ = run(Q, I)
    return out
